# revision 26
# baseline (speedup 1.0000x reference)
"""Trainium2 Bass kernel for nn_Attn_25409026523783.

Dense causal multi-head attention block (B=64, S=256, D=2048, H=16, HD=128):
    qkv = x @ w_qkv.T ; causal softmax attention per head ; out = ctx @ w_o.T

Strategy (fused bf16, zero DRAM spill; PE ~98% busy at the 1 cycle/row
bf16 stream roofline):
  - Batch-shard across the 8 NeuronCores (8 batches / 2048 tokens per core).
    No collectives: host scatters inputs, concatenates per-core outputs.
  - All operands cast to bf16 on the host and pre-tiled so every DMA is a
    contiguous block with the contraction dim on partitions. Matmuls run
    bf16 x bf16 -> fp32 PSUM (1 cycle/row at N>=256, measured same speed as
    fp32r, but half the DMA/SBUF footprint and cheaper LDWEIGHTS).
  - PE warm-up: ~76 junk matmuls on a memset tile (no DMA dependency)
    cover the ~8us DMA-ring spin-up + ~15us first-operand load, so HAM
    un-throttles to 2.4 GHz before the real stream starts.
  - Head-major loop with x^T resident in SBUF (64 KiB/partition bf16,
    loaded in 4 token-major chunks so head 0's projections start early):
    per head, project Q^T/K^T (feature-major) and V (token-major, per
    4-head group), run causal attention for all 8 local batches, and
    write ctx^T into a resident bf16 buffer. Q/K/V never touch DRAM; the
    Tile scheduler interleaves head h's attention with head h+1's
    projections (next head's wqk DMA is prefetched before attention).
  - Attention per (batch, head) in transposed layout S^T[k, q] with causal
    skip (k-tile 1 only computed for queries >= 128), software-pipelined:
    scores/exp(ACT)/mask(DVE) run 2 batches ahead of the dependent work so
    the PE never waits on the softmax chain. Denominators via ONE matmul
    with the all-ones 128x128 mask block as stationary — every output row
    equals the column sums, so the matmul also performs the partition
    broadcast (no GpSimd op, no M=1 col_grp penalty). ctx^T (V x A^T) is
    evacuated unnormalised via ScalarE so its PSUM bank frees immediately;
    a trailing DVE multiply applies 1/den in SBUF off the critical path.
  - The last head has no next-head projections to hide the softmax chain,
    so its attention pipeline is woven between its own Q/K chunk matmuls.
  - Loop-order notes from HW traces: accumulation chains must stay dt-inner
    (one PSUM bank per chain — bank-cycling between consecutive matmuls
    slows the PE ~18%), and gpsimd.partition_all_reduce for the denominators
    is a net loss (~2.8 us/op stalls the pipeline).
  - After the head loop x^T is freed; w_o^T streams in per-512-column
    chunks (split into 4 head-group DMAs so the first output chain starts
    ~1us after the pool opens) and the output projection accumulates ctx^T
    tiles over heads.
"""

import os
import sys

import numpy as np
import ml_dtypes

for _p in ("/opt/trn_rl_repo", "/root/.axon_site/_ro/trn_rl_repo"):
    if os.path.isdir(_p) and _p not in sys.path:
        sys.path.insert(0, _p)

import concourse.bass as bass  # noqa: E402,F401
import concourse.mybir as mybir  # noqa: E402
import concourse.tile as tile  # noqa: E402
from concourse import bacc  # noqa: E402
from concourse.bass_utils import run_bass_kernel_spmd  # noqa: E402


def _ensure_ntff_hook():
    """Some images ship an ``antenv`` without ``axon_hooks``; bass_utils then
    crashes on import when tracing is requested. Provide a no-op-compatible
    module (and register the real ctypes hook when available)."""
    try:
        from antenv import axon_hooks  # noqa: F401
        return
    except ImportError:
        pass
    import types

    mod = types.ModuleType("antenv.axon_hooks")
    mod._hook = None
    mod.set_axon_ntff_profile_hook = lambda h: setattr(mod, "_hook", h)
    mod.get_axon_ntff_profile_hook = lambda: mod._hook
    sys.modules["antenv.axon_hooks"] = mod
    try:
        import antenv

        antenv.axon_hooks = mod
        from trn_agent_boot.trn_boot import _ntff_profile_via_ctypes

        hook = _ntff_profile_via_ctypes("/opt/axon/libaxon_pjrt.so")
        if hook is not None:
            mod._hook = hook
    except Exception:
        pass


_ensure_ntff_hook()

F32 = mybir.dt.float32
F32R = mybir.dt.float32r
BF16 = mybir.dt.bfloat16
EXP = mybir.ActivationFunctionType.Exp

# Problem constants (per spec, hardcoded)
B, S, D, H = 64, 256, 2048, 16
HD = D // H  # 128
N_CORES = 8
NB = B // N_CORES           # 8 batches per core
T = NB * S                  # 2048 tokens per core
P = 128
DT = D // P                 # 16 d-tiles
SCALE = float(HD) ** -0.5
G = 4                       # heads per V-production group

_CACHE = {}


def _build():
    nc = bacc.Bacc("TRN2", target_bir_lowering=False, debug=False,
                   enable_asserts=False)

    # x^T in token-major chunks: [p(d), tch, dt, 512] so each 512-token block
    # is one contiguous DMA and head-0 projections can start after 1/4 of x.
    xt_t = nc.dram_tensor("xt_t", [P, T // 512, DT, 512], BF16,
                          kind="ExternalInput").ap()
    # per-head Q|K weight stripes: [h, p(d), dt, 256] (q cols 0:128, k 128:256)
    wqk_t = nc.dram_tensor("wqk_t", [H, P, DT, 256], BF16,
                           kind="ExternalInput").ap()
    # V weight stripes per 4-head group: [g, p(d), dt, 512]
    wv_t = nc.dram_tensor("wv_t", [G, P, DT, 512], BF16,
                          kind="ExternalInput").ap()
    # w_o^T stripes per 512-col output chunk: [ec, p(d'), h, 512]
    wo_t = nc.dram_tensor("wo_t", [D // 512, P, H, 512], BF16,
                          kind="ExternalInput").ap()
    # mask2 [128, 384]: [tri(k<=q) | ones | tri] (causal-skip layout)
    mask2 = nc.dram_tensor("mask2", [P, 3 * P], BF16,
                           kind="ExternalInput").ap()
    out = nc.dram_tensor("out", [T, D], F32, kind="ExternalOutput").ap()

    with tile.TileContext(nc) as tc:
        with tc.tile_pool(name="const", bufs=1) as c_pool, \
             tc.tile_pool(name="ctx", bufs=1) as ctx_pool:

            m2 = c_pool.tile([P, 3 * P], BF16)
            nc.sync.dma_start(out=m2[:], in_=mask2)
            ones_col = m2[:, 2 * P - 1:2 * P]   # all-ones [128, 1] bf16
            ones_row = m2[0:1, P:2 * P]         # all-ones [1, 128] bf16

            # resident ctx^T accumulator: [128 d', 16 h, 2048 t] bf16
            ctx_b = ctx_pool.tile([P, H, T], BF16, name="ctx_b")

            # PE warm-up: throwaway matmuls on a memset junk tile (no DMA
            # dependency — the DMA rings take ~8us to start moving data and
            # the first real chain needs ~3MB in SBUF, i.e. ~15us). This
            # keeps the PE busy from ~0.5us so HAM un-throttles early and
            # the real stream starts at full 2.4 GHz with zero idle.
            junk = c_pool.tile([P, 384], BF16, name="warm_junk")
            nc.gpsimd.memset(junk[:], 0)
            with tc.tile_pool(name="warm", bufs=1, space="PSUM") as w_ps:
                wps = w_ps.tile([P, 384], F32)
                for _ in range(76):
                    nc.tensor.matmul(wps[:], junk[:, 0:P], junk[:],
                                     start=True, stop=True)

            # ---------------- head loop: QKV + attention ----------------
            with tc.tile_pool(name="xt", bufs=1) as xt_pool, \
                 tc.tile_pool(name="wqk", bufs=2) as wqk_pool, \
                 tc.tile_pool(name="wv", bufs=1) as wv_pool, \
                 tc.tile_pool(name="qk", bufs=2) as qk_pool, \
                 tc.tile_pool(name="vg", bufs=1) as v_pool, \
                 tc.tile_pool(name="at", bufs=4) as a_pool, \
                 tc.tile_pool(name="rcp", bufs=3) as r_pool, \
                 tc.tile_pool(name="ps_qkv", bufs=3, space="PSUM") as qkv_ps, \
                 tc.tile_pool(name="ps_s", bufs=3, space="PSUM") as s_ps, \
                 tc.tile_pool(name="ps_c", bufs=2, space="PSUM") as c_ps:

                # head-0 weights first so the PE can start a few us in; x^T
                # streams behind them in 4 token-major chunks so head-0's
                # projections unblock chunk by chunk (the warm-up matmuls
                # above cover the first ~9us of DMA).
                wqk0 = wqk_pool.tile([P, DT, 256], BF16, tag="wqk")
                nc.sync.dma_start(out=wqk0[:], in_=wqk_t[0])

                xt = xt_pool.tile([P, T // 512, DT, 512], BF16)
                nc.sync.dma_start(out=xt[:, 0], in_=xt_t[:, 0])

                wv0 = wv_pool.tile([P, DT, 512], BF16, tag="wv")
                nc.sync.dma_start(out=wv0[:], in_=wv_t[0])

                for tch in range(1, T // 512):
                    nc.sync.dma_start(out=xt[:, tch], in_=xt_t[:, tch])

                copy_state = [0]

                def psum_out(dst, ps):
                    if copy_state[0] % 2 == 0:
                        nc.vector.tensor_copy(dst, ps)
                    else:
                        nc.scalar.copy(dst, ps)
                    copy_state[0] += 1

                def emit_qk_chunk(wqk, qk, half, tch, tag):
                    ps = qkv_ps.tile([P, 512], F32, tag="qps",
                                     name=f"qps{tag}{half}{tch}")
                    for dt_ in range(DT):
                        nc.tensor.matmul(
                            ps[:],
                            wqk[:, dt_, half * P:(half + 1) * P],
                            xt[:, tch, dt_, :],
                            start=(dt_ == 0), stop=(dt_ == DT - 1),
                        )
                    psum_out(qk[:, half, tch * 512:(tch + 1) * 512], ps[:])

                def emit_v_chunk(wv, vg, tt, tag):
                    ps = qkv_ps.tile([P, 512], F32, tag="qps",
                                     name=f"vps{tag}{tt}")
                    for dt_ in range(DT):
                        nc.tensor.matmul(
                            ps[:],
                            xt[:, tt // 4, dt_, (tt % 4) * P:(tt % 4 + 1) * P],
                            wv[:, dt_, :],
                            start=(dt_ == 0), stop=(dt_ == DT - 1),
                        )
                    psum_out(vg[:, tt, :], ps[:])

                vg = None
                wqk_next = wqk0
                for h in range(H):
                    hh = h % G
                    # -- weights for this head (and V group) --
                    wqk = wqk_next
                    if hh == 0:
                        vg = v_pool.tile([P, T // P, 512], BF16, tag="vg")
                        if h == 0:
                            wv = wv0
                        else:
                            wv = wv_pool.tile([P, DT, 512], BF16, tag="wv")
                            nc.sync.dma_start(out=wv[:], in_=wv_t[h // G])

                    # -- Q^T / K^T projection: [d', t] feature-major.
                    # dt-inner: each accumulation chain stays in one PSUM
                    # bank (bank-cycling between consecutive MMs measurably
                    # slows the PE).
                    qk = qk_pool.tile([P, 2, T], BF16, tag="qk")
                    if h == 0:
                        # token-chunk-major so each x^T DMA chunk unblocks
                        # the next slice of projection work immediately.
                        for tch in range(T // 512):
                            emit_qk_chunk(wqk, qk, 0, tch, h)
                            emit_qk_chunk(wqk, qk, 1, tch, h)
                            for tt in range(4 * tch, 4 * tch + 4):
                                emit_v_chunk(wv, vg, tt, h // G)
                    elif h < H - 1:
                        for half in range(2):
                            for tch in range(T // 512):
                                emit_qk_chunk(wqk, qk, half, tch, h)
                        if hh == 0:
                            for tt in range(T // P):
                                emit_v_chunk(wv, vg, tt, h // G)
                    # (the last head's Q/K chunks are emitted inside the
                    # attention weave below — they are its only PE filler)

                    # prefetch the next head's Q|K weights before the
                    # attention block so the DMA overlaps it (the first
                    # Q chunk of head h+1 otherwise stalls on this load).
                    if h + 1 < H:
                        wqk_next = wqk_pool.tile([P, DT, 256], BF16,
                                                 tag="wqk")
                        nc.sync.dma_start(out=wqk_next[:], in_=wqk_t[h + 1])

                    # -- causal attention for the 8 local batches.
                    # k-tile 1 only attends to queries q>=128 (causal skip):
                    # a_t columns [0:256] are k-tile 0 x all q, [256:384]
                    # are k-tile 1 x q in [128, 256).
                    def attn_a(b):
                        """scores + exp + mask -> a_t for batch b."""
                        t0 = b * S
                        ps_s = s_ps.tile([P, S + P], F32)
                        nc.tensor.matmul(
                            ps_s[:, 0:S], qk[:, 1, t0:t0 + P],
                            qk[:, 0, t0:t0 + S], start=True, stop=True)
                        nc.tensor.matmul(
                            ps_s[:, S:S + P], qk[:, 1, t0 + P:t0 + S],
                            qk[:, 0, t0 + P:t0 + S], start=True, stop=True)
                        a_t = a_pool.tile([P, S + P], BF16, tag="at")
                        nc.scalar.activation(a_t[:], ps_s[:], EXP, scale=SCALE)
                        nc.vector.tensor_mul(a_t[:], a_t[:], m2[:])
                        return a_t, ps_s

                    def attn_b(b, a_t, ps_s):
                        """unnormalised ctx^T + denominators for batch b."""
                        t0 = b * S
                        # ctx^T over the two k-tiles; evacuate via ScalarE
                        # immediately (no rb dependency) so the PSUM bank
                        # frees fast — normalisation happens in SBUF later.
                        ps_c = c_ps.tile([P, S], F32)
                        nc.tensor.matmul(
                            ps_c[:], vg[:, 2 * b, hh * P:(hh + 1) * P],
                            a_t[:, 0:S], start=True, stop=False)
                        nc.tensor.matmul(
                            ps_c[:, P:S], vg[:, 2 * b + 1, hh * P:(hh + 1) * P],
                            a_t[:, S:S + P], start=False, stop=True,
                            skip_group_check=True)
                        nc.scalar.copy(ctx_b[:, h, t0:t0 + S], ps_c[:])
                        # denominators via two accumulating ones-column
                        # (M=1) matmuls: the k-tile-1 partials accumulate
                        # straight onto the q>=128 denominators in PSUM, so
                        # the DVE chain is just reciprocal (no copy/add —
                        # a congested DVE delays later pairs' mask-muls and
                        # stalls the PE). M=1 keeps PE array duty low: the
                        # full-M all-ones variant tips the chip into its
                        # ~2.0 GHz P0 power state (whole-run +20%). The
                        # output overwrites this pair's scores bank (free
                        # after exp) — no separate PSUM pool, which buys a
                        # third bank for both the scores and QKV chains.
                        nc.tensor.matmul(ps_s[0:1, 0:S], ones_col,
                                         a_t[:, 0:S],
                                         start=True, stop=False)
                        nc.tensor.matmul(ps_s[0:1, P:S], ones_col,
                                         a_t[:, S:S + P],
                                         start=False, stop=True,
                                         skip_group_check=True)
                        rcp = r_pool.tile([1, S], F32, tag="rcp")
                        nc.vector.reciprocal_approx_fast(
                            rcp[:], ps_s[0:1, 0:S])
                        # broadcast 1/denom across partitions on GpSimd
                        rb = r_pool.tile([P, S], F32, tag="rb")
                        nc.gpsimd.partition_broadcast(rb[:], rcp[:],
                                                      channels=P)
                        return rb

                    def attn_norm(b, rb):
                        """ctx_b[h, batch b] *= 1/denom (in SBUF)."""
                        t0 = b * S
                        nc.vector.tensor_mul(ctx_b[:, h, t0:t0 + S],
                                             ctx_b[:, h, t0:t0 + S], rb[:])

                    # Software-pipelined: the scores/exp/mask stage runs 2
                    # batches ahead of the dependent ctx/denominator stage,
                    # so the PE never waits on the ACT+DVE softmax chain;
                    # the normalisation (behind the 0.7us GpSimd broadcast)
                    # trails one more batch, off the critical path.
                    live, norm = [], []

                    def attn_step(b):
                        live.append((b,) + attn_a(b))
                        if len(live) > 2:
                            b0, a0, s0 = live.pop(0)
                            norm.append((b0, attn_b(b0, a0, s0)))
                        while len(norm) > 1:
                            attn_norm(*norm.pop(0))

                    if h < H - 1:
                        for b in range(NB):
                            attn_step(b)
                    else:
                        # last head: no next-head projections exist as PE
                        # filler, so weave the pipeline between this head's
                        # own Q/K chunks (pairs for token block tch only
                        # need the qk chunks of block b//2, so they trail
                        # the chunk loop by one block).
                        for tch in range(T // 512):
                            emit_qk_chunk(wqk, qk, 0, tch, h)
                            emit_qk_chunk(wqk, qk, 1, tch, h)
                            if tch >= 1:
                                attn_step(2 * (tch - 1))
                                attn_step(2 * tch - 1)
                        attn_step(NB - 2)
                        attn_step(NB - 1)
                    for b0, a0, s0 in live:
                        norm.append((b0, attn_b(b0, a0, s0)))
                    for item in norm:
                        attn_norm(*item)

            # ---------------- output projection ----------------
            with tc.tile_pool(name="wo", bufs=2) as wo_pool, \
                 tc.tile_pool(name="p3out", bufs=4) as o3_pool, \
                 tc.tile_pool(name="ps_o", bufs=2, space="PSUM") as o_ps:
                copy_i = 0
                for ec in range(D // 512):
                    wo = wo_pool.tile([P, H, 512], BF16, tag="wo")
                    # split by head group: the first accumulation chain only
                    # needs wo[:, 0:4] — it can start ~1.2us after the DMA
                    # ring picks this up instead of waiting for all 2 MiB.
                    for gq in range(4):
                        nc.sync.dma_start(out=wo[:, 4 * gq:4 * gq + 4, :],
                                          in_=wo_t[ec, :, 4 * gq:4 * gq + 4, :])
                    for tt in range(T // P):
                        ps_o = o_ps.tile([P, 512], F32)
                        for h in range(H):
                            nc.tensor.matmul(
                                ps_o[:],
                                ctx_b[:, h, tt * P:(tt + 1) * P],
                                wo[:, h, :],
                                start=(h == 0), stop=(h == H - 1),
                            )
                        o_t = o3_pool.tile([P, 512], F32, tag="o3")
                        if copy_i % 2 == 0:
                            nc.vector.tensor_copy(o_t[:], ps_o[:])
                        else:
                            nc.scalar.copy(o_t[:], ps_o[:])
                        copy_i += 1
                        nc.sync.dma_start(
                            out=out[tt * P:(tt + 1) * P,
                                    ec * 512:(ec + 1) * 512],
                            in_=o_t[:],
                        )

    nc.compile()
    return nc


def get_nc():
    if "nc" not in _CACHE:
        _CACHE["nc"] = _build()
    return _CACHE["nc"]


def make_in_maps(x, w_qkv, w_o):
    x = np.ascontiguousarray(np.asarray(x, dtype=np.float32))
    w_qkv = np.asarray(w_qkv, dtype=np.float32)
    w_o = np.asarray(w_o, dtype=np.float32)
    bf = ml_dtypes.bfloat16
    # wqk_t [H, P, DT, 256]: [h,p,dt,j<128] = w_qkv[h*128+j, dt*128+p]
    wq = w_qkv[0:D].reshape(H, HD, DT, P).transpose(0, 3, 2, 1)
    wk = w_qkv[D:2 * D].reshape(H, HD, DT, P).transpose(0, 3, 2, 1)
    wqk = np.ascontiguousarray(
        np.concatenate([wq, wk], axis=3)).astype(bf)
    # wv_t [G, P, DT, 512]: [g,p,dt,j] = w_qkv[2D + g*512 + j, dt*128+p]
    wv = np.ascontiguousarray(
        w_qkv[2 * D:].reshape(G, 512, DT, P).transpose(0, 3, 2, 1)).astype(bf)
    # wo_t [EC, P, H, 512]: [ec,p,h,j] = w_o[ec*512+j, h*128+p]
    wo = np.ascontiguousarray(
        w_o.reshape(D // 512, 512, H, HD).transpose(0, 3, 2, 1)).astype(bf)
    # causal mask blocks: [tri(k<=q) | ones | tri]
    tri = np.triu(np.ones((P, P), dtype=np.float32))
    mask2 = np.concatenate(
        [tri, np.ones((P, P), np.float32), tri], axis=1).astype(bf)
    in_maps = []
    for c in range(N_CORES):
        xs = x[c * NB:(c + 1) * NB].reshape(T, D)
        # [P, tch, DT, 512]: [p, tch, dt, j] = xs[tch*512 + j, dt*128 + p]
        xt = np.ascontiguousarray(
            xs.reshape(T // 512, 512, DT, P).transpose(3, 0, 2, 1)).astype(bf)
        in_maps.append({"xt_t": xt, "wqk_t": wqk, "wv_t": wv, "wo_t": wo,
                        "mask2": mask2})
    return in_maps


def run(x, w_qkv, w_o, trace=False):
    nc = get_nc()
    in_maps = make_in_maps(x, w_qkv, w_o)
    res = run_bass_kernel_spmd(nc, in_maps, list(range(N_CORES)), trace=trace)
    outs = [res.results[i]["out"].reshape(NB, S, D) for i in range(N_CORES)]
    return np.concatenate(outs, axis=0), res


def kernel(**inputs):
    out, _ = run(inputs["x"], inputs["w_qkv"], inputs["w_o"])
    return out



# revision 28
# speedup vs baseline: 1.2178x; 1.2178x over previous
"""Trainium2 Bass kernel for nn_Attn_25409026523783.

Dense causal multi-head attention block (B=64, S=256, D=2048, H=16, HD=128):
    qkv = x @ w_qkv.T ; causal softmax attention per head ; out = ctx @ w_o.T

Strategy (fused bf16, zero DRAM spill; PE ~98% busy at the 1 cycle/row
bf16 stream roofline):
  - Batch-shard across the 8 NeuronCores (8 batches / 2048 tokens per core).
    No collectives: host scatters inputs, concatenates per-core outputs.
  - All operands cast to bf16 on the host and pre-tiled so every DMA is a
    contiguous block with the contraction dim on partitions. Matmuls run
    bf16 x bf16 -> fp32 PSUM (1 cycle/row at N>=256, measured same speed as
    fp32r, but half the DMA/SBUF footprint and cheaper LDWEIGHTS).
  - PE warm-up: ~76 junk matmuls on a memset tile (no DMA dependency)
    cover the ~8us DMA-ring spin-up + ~15us first-operand load, so HAM
    un-throttles to 2.4 GHz before the real stream starts.
  - Head-major loop with x^T resident in SBUF (64 KiB/partition bf16,
    loaded in 4 token-major chunks so head 0's projections start early):
    per head, project Q^T/K^T (feature-major) and V (token-major, per
    4-head group), run causal attention for all 8 local batches, and
    write ctx^T into a resident bf16 buffer. Q/K/V never touch DRAM; the
    Tile scheduler interleaves head h's attention with head h+1's
    projections (next head's wqk DMA is prefetched before attention).
  - Attention per (batch, head) in transposed layout S^T[k, q] with causal
    skip (k-tile 1 only computed for queries >= 128), software-pipelined:
    scores/exp(ACT)/mask(DVE) run 2 batches ahead of the dependent work so
    the PE never waits on the softmax chain. Denominators via ONE matmul
    with the all-ones 128x128 mask block as stationary — every output row
    equals the column sums, so the matmul also performs the partition
    broadcast (no GpSimd op, no M=1 col_grp penalty). ctx^T (V x A^T) is
    evacuated unnormalised via ScalarE so its PSUM bank frees immediately;
    a trailing DVE multiply applies 1/den in SBUF off the critical path.
  - The last head has no next-head projections to hide the softmax chain,
    so its attention pipeline is woven between its own Q/K chunk matmuls.
  - Loop-order notes from HW traces: accumulation chains must stay dt-inner
    (one PSUM bank per chain — bank-cycling between consecutive matmuls
    slows the PE ~18%), and gpsimd.partition_all_reduce for the denominators
    is a net loss (~2.8 us/op stalls the pipeline).
  - After the head loop x^T is freed; w_o^T streams in per-512-column
    chunks (split into 4 head-group DMAs so the first output chain starts
    ~1us after the pool opens) and the output projection accumulates ctx^T
    tiles over heads.
"""

import os
import sys

import numpy as np
import ml_dtypes

for _p in ("/opt/trn_rl_repo", "/root/.axon_site/_ro/trn_rl_repo"):
    if os.path.isdir(_p) and _p not in sys.path:
        sys.path.insert(0, _p)

import concourse.bass as bass  # noqa: E402,F401
import concourse.mybir as mybir  # noqa: E402
import concourse.tile as tile  # noqa: E402
from concourse import bacc  # noqa: E402
from concourse.bass_utils import run_bass_kernel_spmd  # noqa: E402


def _ensure_ntff_hook():
    """Some images ship an ``antenv`` without ``axon_hooks``; bass_utils then
    crashes on import when tracing is requested. Provide a no-op-compatible
    module (and register the real ctypes hook when available)."""
    try:
        from antenv import axon_hooks  # noqa: F401
        return
    except ImportError:
        pass
    import types

    mod = types.ModuleType("antenv.axon_hooks")
    mod._hook = None
    mod.set_axon_ntff_profile_hook = lambda h: setattr(mod, "_hook", h)
    mod.get_axon_ntff_profile_hook = lambda: mod._hook
    sys.modules["antenv.axon_hooks"] = mod
    try:
        import antenv

        antenv.axon_hooks = mod
        from trn_agent_boot.trn_boot import _ntff_profile_via_ctypes

        hook = _ntff_profile_via_ctypes("/opt/axon/libaxon_pjrt.so")
        if hook is not None:
            mod._hook = hook
    except Exception:
        pass


_ensure_ntff_hook()

F32 = mybir.dt.float32
F32R = mybir.dt.float32r
BF16 = mybir.dt.bfloat16
EXP = mybir.ActivationFunctionType.Exp

# Problem constants (per spec, hardcoded)
B, S, D, H = 64, 256, 2048, 16
HD = D // H  # 128
N_CORES = 8
NB = B // N_CORES           # 8 batches per core
T = NB * S                  # 2048 tokens per core
P = 128
DT = D // P                 # 16 d-tiles
SCALE = float(HD) ** -0.5
G = 4                       # heads per V-production group

_CACHE = {}


def _build():
    nc = bacc.Bacc("TRN2", target_bir_lowering=False, debug=False,
                   enable_asserts=False)

    # x^T in token-major chunks: [p(d), tch, dt, 512] so each 512-token block
    # is one contiguous DMA and head-0 projections can start after 1/4 of x.
    xt_t = nc.dram_tensor("xt_t", [P, T // 512, DT, 512], BF16,
                          kind="ExternalInput").ap()
    # per-head Q|K weight stripes: [h, p(d), dt, 256] (q cols 0:128, k 128:256)
    wqk_t = nc.dram_tensor("wqk_t", [H, P, DT, 256], BF16,
                           kind="ExternalInput").ap()
    # V weight stripes per 4-head group: [g, p(d), dt, 512]
    wv_t = nc.dram_tensor("wv_t", [G, P, DT, 512], BF16,
                          kind="ExternalInput").ap()
    # w_o^T stripes per 512-col output chunk: [ec, p(d'), h, 512]
    wo_t = nc.dram_tensor("wo_t", [D // 512, P, H, 512], BF16,
                          kind="ExternalInput").ap()
    # mask2 [128, 384]: [tri(k<=q) | ones | tri] (causal-skip layout)
    mask2 = nc.dram_tensor("mask2", [P, 3 * P], BF16,
                           kind="ExternalInput").ap()
    out = nc.dram_tensor("out", [T, D], F32, kind="ExternalOutput").ap()

    with tile.TileContext(nc) as tc:
        with tc.tile_pool(name="const", bufs=1) as c_pool, \
             tc.tile_pool(name="ctx", bufs=1) as ctx_pool:

            m2 = c_pool.tile([P, 3 * P], BF16)
            nc.sync.dma_start(out=m2[:], in_=mask2)
            ones_col = m2[:, 2 * P - 1:2 * P]   # all-ones [128, 1] bf16
            ones_row = m2[0:1, P:2 * P]         # all-ones [1, 128] bf16

            # resident ctx^T accumulator: [128 d', 16 h, 2048 t] bf16
            ctx_b = ctx_pool.tile([P, H, T], BF16, name="ctx_b")

            # PE warm-up: throwaway matmuls on a memset junk tile (no DMA
            # dependency — the DMA rings take ~8us to start moving data and
            # the first real chain needs ~3MB in SBUF, i.e. ~15us). This
            # keeps the PE busy from ~0.5us so HAM un-throttles early and
            # the real stream starts at full 2.4 GHz with zero idle.
            junk = c_pool.tile([P, 384], BF16, name="warm_junk")
            nc.gpsimd.memset(junk[:], 0)
            with tc.tile_pool(name="warm", bufs=1, space="PSUM") as w_ps:
                wps = w_ps.tile([P, 384], F32)
                for _ in range(76):
                    nc.tensor.matmul(wps[:], junk[:, 0:P], junk[:],
                                     start=True, stop=True)

            # ---------------- head loop: QKV + attention ----------------
            with tc.tile_pool(name="xt", bufs=1) as xt_pool, \
                 tc.tile_pool(name="wqk", bufs=2) as wqk_pool, \
                 tc.tile_pool(name="wv", bufs=1) as wv_pool, \
                 tc.tile_pool(name="qk", bufs=2) as qk_pool, \
                 tc.tile_pool(name="vg", bufs=1) as v_pool, \
                 tc.tile_pool(name="at", bufs=4) as a_pool, \
                 tc.tile_pool(name="rcp", bufs=3) as r_pool, \
                 tc.tile_pool(name="ps_qkv", bufs=2, space="PSUM") as qkv_ps, \
                 tc.tile_pool(name="ps_s", bufs=2, space="PSUM") as s_ps, \
                 tc.tile_pool(name="ps_db", bufs=2, space="PSUM") as db_ps, \
                 tc.tile_pool(name="ps_c", bufs=2, space="PSUM") as c_ps:

                # head-0 weights first so the PE can start a few us in; x^T
                # streams behind them in 4 token-major chunks so head-0's
                # projections unblock chunk by chunk (the warm-up matmuls
                # above cover the first ~9us of DMA).
                wqk0 = wqk_pool.tile([P, DT, 256], BF16, tag="wqk")
                nc.sync.dma_start(out=wqk0[:], in_=wqk_t[0])

                xt = xt_pool.tile([P, T // 512, DT, 512], BF16)
                nc.sync.dma_start(out=xt[:, 0], in_=xt_t[:, 0])

                wv0 = wv_pool.tile([P, DT, 512], BF16, tag="wv")
                nc.sync.dma_start(out=wv0[:], in_=wv_t[0])

                for tch in range(1, T // 512):
                    nc.sync.dma_start(out=xt[:, tch], in_=xt_t[:, tch])

                copy_state = [0]

                def psum_out(dst, ps):
                    if copy_state[0] % 2 == 0:
                        nc.vector.tensor_copy(dst, ps)
                    else:
                        nc.scalar.copy(dst, ps)
                    copy_state[0] += 1

                def emit_qk_chunk(wqk, qk, half, tch, tag):
                    ps = qkv_ps.tile([P, 512], F32, tag="qps",
                                     name=f"qps{tag}{half}{tch}")
                    for dt_ in range(DT):
                        nc.tensor.matmul(
                            ps[:],
                            wqk[:, dt_, half * P:(half + 1) * P],
                            xt[:, tch, dt_, :],
                            start=(dt_ == 0), stop=(dt_ == DT - 1),
                        )
                    psum_out(qk[:, half, tch * 512:(tch + 1) * 512], ps[:])

                def emit_v_chunk(wv, vg, tt, tag):
                    ps = qkv_ps.tile([P, 512], F32, tag="qps",
                                     name=f"vps{tag}{tt}")
                    for dt_ in range(DT):
                        nc.tensor.matmul(
                            ps[:],
                            xt[:, tt // 4, dt_, (tt % 4) * P:(tt % 4 + 1) * P],
                            wv[:, dt_, :],
                            start=(dt_ == 0), stop=(dt_ == DT - 1),
                        )
                    psum_out(vg[:, tt, :], ps[:])

                vg = None
                wqk_next = wqk0
                for h in range(H):
                    hh = h % G
                    # -- weights for this head (and V group) --
                    wqk = wqk_next
                    if hh == 0:
                        vg = v_pool.tile([P, T // P, 512], BF16, tag="vg")
                        if h == 0:
                            wv = wv0
                        else:
                            wv = wv_pool.tile([P, DT, 512], BF16, tag="wv")
                            nc.sync.dma_start(out=wv[:], in_=wv_t[h // G])

                    # -- Q^T / K^T projection: [d', t] feature-major.
                    # dt-inner: each accumulation chain stays in one PSUM
                    # bank (bank-cycling between consecutive MMs measurably
                    # slows the PE).
                    qk = qk_pool.tile([P, 2, T], BF16, tag="qk")
                    if h == 0:
                        # token-chunk-major so each x^T DMA chunk unblocks
                        # the next slice of projection work immediately.
                        for tch in range(T // 512):
                            emit_qk_chunk(wqk, qk, 0, tch, h)
                            emit_qk_chunk(wqk, qk, 1, tch, h)
                            for tt in range(4 * tch, 4 * tch + 4):
                                emit_v_chunk(wv, vg, tt, h // G)
                    elif h < H - 1:
                        for half in range(2):
                            for tch in range(T // 512):
                                emit_qk_chunk(wqk, qk, half, tch, h)
                        if hh == 0:
                            for tt in range(T // P):
                                emit_v_chunk(wv, vg, tt, h // G)
                    # (the last head's Q/K chunks are emitted inside the
                    # attention weave below — they are its only PE filler)

                    # prefetch the next head's Q|K weights before the
                    # attention block so the DMA overlaps it (the first
                    # Q chunk of head h+1 otherwise stalls on this load).
                    if h + 1 < H:
                        wqk_next = wqk_pool.tile([P, DT, 256], BF16,
                                                 tag="wqk")
                        nc.sync.dma_start(out=wqk_next[:], in_=wqk_t[h + 1])

                    # -- causal attention for the 8 local batches.
                    # k-tile 1 only attends to queries q>=128 (causal skip):
                    # a_t columns [0:256] are k-tile 0 x all q, [256:384]
                    # are k-tile 1 x q in [128, 256).
                    def attn_a(b):
                        """scores + exp + mask -> a_t for batch b."""
                        t0 = b * S
                        ps_s = s_ps.tile([P, S + P], F32)
                        nc.tensor.matmul(
                            ps_s[:, 0:S], qk[:, 1, t0:t0 + P],
                            qk[:, 0, t0:t0 + S], start=True, stop=True)
                        nc.tensor.matmul(
                            ps_s[:, S:S + P], qk[:, 1, t0 + P:t0 + S],
                            qk[:, 0, t0 + P:t0 + S], start=True, stop=True)
                        a_t = a_pool.tile([P, S + P], BF16, tag="at")
                        nc.scalar.activation(a_t[:], ps_s[:], EXP, scale=SCALE)
                        nc.vector.tensor_mul(a_t[:], a_t[:], m2[:])
                        return a_t, ps_s

                    def attn_b(b, a_t, ps_s):
                        """unnormalised ctx^T + denominators for batch b."""
                        t0 = b * S
                        # ctx^T over the two k-tiles; evacuate via ScalarE
                        # immediately (no rb dependency) so the PSUM bank
                        # frees fast — normalisation happens in SBUF later.
                        ps_c = c_ps.tile([P, S], F32)
                        nc.tensor.matmul(
                            ps_c[:], vg[:, 2 * b, hh * P:(hh + 1) * P],
                            a_t[:, 0:S], start=True, stop=False)
                        nc.tensor.matmul(
                            ps_c[:, P:S], vg[:, 2 * b + 1, hh * P:(hh + 1) * P],
                            a_t[:, S:S + P], start=False, stop=True,
                            skip_group_check=True)
                        nc.scalar.copy(ctx_b[:, h, t0:t0 + S], ps_c[:])
                        # denominators via ONE matmul with the ALL-ONES
                        # 128x128 mask block as stationary: every output
                        # row equals the column sums, i.e. the matmul does
                        # the partition-broadcast for free (no GpSimd op,
                        # no M=1 col_grp penalty). k-tile-1 partials land
                        # in cols [256:384]; a DVE add (staged via SBUF —
                        # one PSUM read per DVE op) folds them onto the
                        # q>=128 denominators, then the reciprocal writes
                        # the broadcast 1/den tile directly.
                        ps_db = db_ps.tile([P, S + P], F32)
                        nc.tensor.matmul(ps_db[:, 0:S + P], m2[:, P:2 * P],
                                         a_t[:, 0:S + P],
                                         start=True, stop=True)
                        dtmp = r_pool.tile([P, P], F32, tag="dtmp")
                        nc.vector.tensor_copy(dtmp[:], ps_db[:, S:S + P])
                        nc.vector.tensor_add(ps_db[:, P:S],
                                             ps_db[:, P:S], dtmp[:])
                        rb = r_pool.tile([P, S], F32, tag="rb")
                        nc.vector.reciprocal_approx_fast(
                            rb[:], ps_db[:, 0:S])
                        return rb

                    def attn_norm(b, rb):
                        """ctx_b[h, batch b] *= 1/denom (in SBUF)."""
                        t0 = b * S
                        nc.vector.tensor_mul(ctx_b[:, h, t0:t0 + S],
                                             ctx_b[:, h, t0:t0 + S], rb[:])

                    # Software-pipelined: the scores/exp/mask stage runs 2
                    # batches ahead of the dependent ctx/denominator stage,
                    # so the PE never waits on the ACT+DVE softmax chain;
                    # the normalisation (behind the 0.7us GpSimd broadcast)
                    # trails one more batch, off the critical path.
                    live, norm = [], []

                    def attn_step(b):
                        live.append((b,) + attn_a(b))
                        if len(live) > 2:
                            b0, a0, s0 = live.pop(0)
                            norm.append((b0, attn_b(b0, a0, s0)))
                        while len(norm) > 1:
                            attn_norm(*norm.pop(0))

                    if h < H - 1:
                        for b in range(NB):
                            attn_step(b)
                    else:
                        # last head: no next-head projections exist as PE
                        # filler, so weave the pipeline between this head's
                        # own Q/K chunks (pairs for token block tch only
                        # need the qk chunks of block b//2, so they trail
                        # the chunk loop by one block).
                        for tch in range(T // 512):
                            emit_qk_chunk(wqk, qk, 0, tch, h)
                            emit_qk_chunk(wqk, qk, 1, tch, h)
                            if tch >= 1:
                                attn_step(2 * (tch - 1))
                                attn_step(2 * tch - 1)
                        attn_step(NB - 2)
                        attn_step(NB - 1)
                    for b0, a0, s0 in live:
                        norm.append((b0, attn_b(b0, a0, s0)))
                    for item in norm:
                        attn_norm(*item)

            # ---------------- output projection ----------------
            with tc.tile_pool(name="wo", bufs=2) as wo_pool, \
                 tc.tile_pool(name="p3out", bufs=4) as o3_pool, \
                 tc.tile_pool(name="ps_o", bufs=2, space="PSUM") as o_ps:
                copy_i = 0
                for ec in range(D // 512):
                    wo = wo_pool.tile([P, H, 512], BF16, tag="wo")
                    # split by head group: the first accumulation chain only
                    # needs wo[:, 0:4] — it can start ~1.2us after the DMA
                    # ring picks this up instead of waiting for all 2 MiB.
                    for gq in range(4):
                        nc.sync.dma_start(out=wo[:, 4 * gq:4 * gq + 4, :],
                                          in_=wo_t[ec, :, 4 * gq:4 * gq + 4, :])
                    for tt in range(T // P):
                        ps_o = o_ps.tile([P, 512], F32)
                        for h in range(H):
                            nc.tensor.matmul(
                                ps_o[:],
                                ctx_b[:, h, tt * P:(tt + 1) * P],
                                wo[:, h, :],
                                start=(h == 0), stop=(h == H - 1),
                            )
                        o_t = o3_pool.tile([P, 512], F32, tag="o3")
                        if copy_i % 2 == 0:
                            nc.vector.tensor_copy(o_t[:], ps_o[:])
                        else:
                            nc.scalar.copy(o_t[:], ps_o[:])
                        copy_i += 1
                        nc.sync.dma_start(
                            out=out[tt * P:(tt + 1) * P,
                                    ec * 512:(ec + 1) * 512],
                            in_=o_t[:],
                        )

    nc.compile()
    return nc


def get_nc():
    if "nc" not in _CACHE:
        _CACHE["nc"] = _build()
    return _CACHE["nc"]


def make_in_maps(x, w_qkv, w_o):
    x = np.ascontiguousarray(np.asarray(x, dtype=np.float32))
    w_qkv = np.asarray(w_qkv, dtype=np.float32)
    w_o = np.asarray(w_o, dtype=np.float32)
    bf = ml_dtypes.bfloat16
    # wqk_t [H, P, DT, 256]: [h,p,dt,j<128] = w_qkv[h*128+j, dt*128+p]
    wq = w_qkv[0:D].reshape(H, HD, DT, P).transpose(0, 3, 2, 1)
    wk = w_qkv[D:2 * D].reshape(H, HD, DT, P).transpose(0, 3, 2, 1)
    wqk = np.ascontiguousarray(
        np.concatenate([wq, wk], axis=3)).astype(bf)
    # wv_t [G, P, DT, 512]: [g,p,dt,j] = w_qkv[2D + g*512 + j, dt*128+p]
    wv = np.ascontiguousarray(
        w_qkv[2 * D:].reshape(G, 512, DT, P).transpose(0, 3, 2, 1)).astype(bf)
    # wo_t [EC, P, H, 512]: [ec,p,h,j] = w_o[ec*512+j, h*128+p]
    wo = np.ascontiguousarray(
        w_o.reshape(D // 512, 512, H, HD).transpose(0, 3, 2, 1)).astype(bf)
    # causal mask blocks: [tri(k<=q) | ones | tri]
    tri = np.triu(np.ones((P, P), dtype=np.float32))
    mask2 = np.concatenate(
        [tri, np.ones((P, P), np.float32), tri], axis=1).astype(bf)
    in_maps = []
    for c in range(N_CORES):
        xs = x[c * NB:(c + 1) * NB].reshape(T, D)
        # [P, tch, DT, 512]: [p, tch, dt, j] = xs[tch*512 + j, dt*128 + p]
        xt = np.ascontiguousarray(
            xs.reshape(T // 512, 512, DT, P).transpose(3, 0, 2, 1)).astype(bf)
        in_maps.append({"xt_t": xt, "wqk_t": wqk, "wv_t": wv, "wo_t": wo,
                        "mask2": mask2})
    return in_maps


def run(x, w_qkv, w_o, trace=False):
    nc = get_nc()
    in_maps = make_in_maps(x, w_qkv, w_o)
    res = run_bass_kernel_spmd(nc, in_maps, list(range(N_CORES)), trace=trace)
    outs = [res.results[i]["out"].reshape(NB, S, D) for i in range(N_CORES)]
    return np.concatenate(outs, axis=0), res


def kernel(**inputs):
    out, _ = run(inputs["x"], inputs["w_qkv"], inputs["w_o"])
    return out



# revision 29
# speedup vs baseline: 1.2331x; 1.0125x over previous
"""Trainium2 Bass kernel for nn_Attn_25409026523783.

Dense causal multi-head attention block (B=64, S=256, D=2048, H=16, HD=128):
    qkv = x @ w_qkv.T ; causal softmax attention per head ; out = ctx @ w_o.T

Strategy (fused bf16, zero DRAM spill; PE ~98% busy at the 1 cycle/row
bf16 stream roofline):
  - Batch-shard across the 8 NeuronCores (8 batches / 2048 tokens per core).
    No collectives: host scatters inputs, concatenates per-core outputs.
  - All operands cast to bf16 on the host and pre-tiled so every DMA is a
    contiguous block with the contraction dim on partitions. Matmuls run
    bf16 x bf16 -> fp32 PSUM (1 cycle/row at N>=256, measured same speed as
    fp32r, but half the DMA/SBUF footprint and cheaper LDWEIGHTS).
  - PE warm-up: ~76 junk matmuls on a memset tile (no DMA dependency)
    cover the ~8us DMA-ring spin-up + ~15us first-operand load, so HAM
    un-throttles to 2.4 GHz before the real stream starts.
  - Head-major loop with x^T resident in SBUF (64 KiB/partition bf16,
    loaded in 4 token-major chunks so head 0's projections start early):
    per head, project Q^T/K^T (feature-major) and V (token-major, per
    4-head group), run causal attention for all 8 local batches, and
    write ctx^T into a resident bf16 buffer. Q/K/V never touch DRAM; the
    Tile scheduler interleaves head h's attention with head h+1's
    projections (next head's wqk DMA is prefetched before attention).
  - Attention per (batch, head) in transposed layout S^T[k, q] with causal
    skip (k-tile 1 only computed for queries >= 128), software-pipelined:
    scores/exp(ACT)/mask(DVE) run 2 batches ahead of the dependent work so
    the PE never waits on the softmax chain. Denominators via ONE matmul
    with the all-ones 128x128 mask block as stationary — every output row
    equals the column sums, so the matmul also performs the partition
    broadcast (no GpSimd op, no M=1 col_grp penalty). ctx^T (V x A^T) is
    evacuated unnormalised via ScalarE so its PSUM bank frees immediately;
    a trailing DVE multiply applies 1/den in SBUF off the critical path.
  - The last head has no next-head projections to hide the softmax chain,
    so its attention pipeline is woven between its own Q/K chunk matmuls.
  - Loop-order notes from HW traces: accumulation chains must stay dt-inner
    (one PSUM bank per chain — bank-cycling between consecutive matmuls
    slows the PE ~18%), and gpsimd.partition_all_reduce for the denominators
    is a net loss (~2.8 us/op stalls the pipeline).
  - After the head loop x^T is freed; w_o^T streams in per-512-column
    chunks (split into 4 head-group DMAs so the first output chain starts
    ~1us after the pool opens) and the output projection accumulates ctx^T
    tiles over heads.
"""

import os
import sys

import numpy as np
import ml_dtypes

for _p in ("/opt/trn_rl_repo", "/root/.axon_site/_ro/trn_rl_repo"):
    if os.path.isdir(_p) and _p not in sys.path:
        sys.path.insert(0, _p)

import concourse.bass as bass  # noqa: E402,F401
import concourse.mybir as mybir  # noqa: E402
import concourse.tile as tile  # noqa: E402
from concourse import bacc  # noqa: E402
from concourse.bass_utils import run_bass_kernel_spmd  # noqa: E402


def _ensure_ntff_hook():
    """Some images ship an ``antenv`` without ``axon_hooks``; bass_utils then
    crashes on import when tracing is requested. Provide a no-op-compatible
    module (and register the real ctypes hook when available)."""
    try:
        from antenv import axon_hooks  # noqa: F401
        return
    except ImportError:
        pass
    import types

    mod = types.ModuleType("antenv.axon_hooks")
    mod._hook = None
    mod.set_axon_ntff_profile_hook = lambda h: setattr(mod, "_hook", h)
    mod.get_axon_ntff_profile_hook = lambda: mod._hook
    sys.modules["antenv.axon_hooks"] = mod
    try:
        import antenv

        antenv.axon_hooks = mod
        from trn_agent_boot.trn_boot import _ntff_profile_via_ctypes

        hook = _ntff_profile_via_ctypes("/opt/axon/libaxon_pjrt.so")
        if hook is not None:
            mod._hook = hook
    except Exception:
        pass


_ensure_ntff_hook()

F32 = mybir.dt.float32
F32R = mybir.dt.float32r
BF16 = mybir.dt.bfloat16
EXP = mybir.ActivationFunctionType.Exp

# Problem constants (per spec, hardcoded)
B, S, D, H = 64, 256, 2048, 16
HD = D // H  # 128
N_CORES = 8
NB = B // N_CORES           # 8 batches per core
T = NB * S                  # 2048 tokens per core
P = 128
DT = D // P                 # 16 d-tiles
SCALE = float(HD) ** -0.5
G = 4                       # heads per V-production group

_CACHE = {}


def _build():
    nc = bacc.Bacc("TRN2", target_bir_lowering=False, debug=False,
                   enable_asserts=False)

    # x^T in token-major chunks: [p(d), tch, dt, 512] so each 512-token block
    # is one contiguous DMA and head-0 projections can start after 1/4 of x.
    xt_t = nc.dram_tensor("xt_t", [P, T // 512, DT, 512], BF16,
                          kind="ExternalInput").ap()
    # per-head Q|K weight stripes: [h, p(d), dt, 256] (q cols 0:128, k 128:256)
    wqk_t = nc.dram_tensor("wqk_t", [H, P, DT, 256], BF16,
                           kind="ExternalInput").ap()
    # V weight stripes per 4-head group: [g, p(d), dt, 512]
    wv_t = nc.dram_tensor("wv_t", [G, P, DT, 512], BF16,
                          kind="ExternalInput").ap()
    # w_o^T stripes per 512-col output chunk: [ec, p(d'), h, 512]
    wo_t = nc.dram_tensor("wo_t", [D // 512, P, H, 512], BF16,
                          kind="ExternalInput").ap()
    # mask2 [128, 384]: [tri(k<=q) | ones | tri] (causal-skip layout)
    mask2 = nc.dram_tensor("mask2", [P, 3 * P], BF16,
                           kind="ExternalInput").ap()
    out = nc.dram_tensor("out", [T, D], F32, kind="ExternalOutput").ap()

    with tile.TileContext(nc) as tc:
        with tc.tile_pool(name="const", bufs=1) as c_pool, \
             tc.tile_pool(name="ctx", bufs=1) as ctx_pool:

            m2 = c_pool.tile([P, 3 * P], BF16)
            nc.sync.dma_start(out=m2[:], in_=mask2)
            ones_col = m2[:, 2 * P - 1:2 * P]   # all-ones [128, 1] bf16
            ones_row = m2[0:1, P:2 * P]         # all-ones [1, 128] bf16

            # resident ctx^T accumulator: [128 d', 16 h, 2048 t] bf16
            ctx_b = ctx_pool.tile([P, H, T], BF16, name="ctx_b")

            # PE warm-up: throwaway matmuls on a memset junk tile (no DMA
            # dependency — the DMA rings take ~8us to start moving data and
            # the first real chain needs ~3MB in SBUF, i.e. ~15us). This
            # keeps the PE busy from ~0.5us so HAM un-throttles early and
            # the real stream starts at full 2.4 GHz with zero idle.
            junk = c_pool.tile([P, 384], BF16, name="warm_junk")
            nc.gpsimd.memset(junk[:], 0)
            with tc.tile_pool(name="warm", bufs=1, space="PSUM") as w_ps:
                wps = w_ps.tile([P, 384], F32)
                for _ in range(76):
                    nc.tensor.matmul(wps[:], junk[:, 0:P], junk[:],
                                     start=True, stop=True)

            # ---------------- head loop: QKV + attention ----------------
            with tc.tile_pool(name="xt", bufs=1) as xt_pool, \
                 tc.tile_pool(name="wqk", bufs=2) as wqk_pool, \
                 tc.tile_pool(name="wv", bufs=1) as wv_pool, \
                 tc.tile_pool(name="qk", bufs=2) as qk_pool, \
                 tc.tile_pool(name="vg", bufs=1) as v_pool, \
                 tc.tile_pool(name="at", bufs=4) as a_pool, \
                 tc.tile_pool(name="rcp", bufs=3) as r_pool, \
                 tc.tile_pool(name="ps_qkv", bufs=2, space="PSUM") as qkv_ps, \
                 tc.tile_pool(name="ps_s", bufs=2, space="PSUM") as s_ps, \
                 tc.tile_pool(name="ps_db", bufs=2, space="PSUM") as db_ps, \
                 tc.tile_pool(name="ps_c", bufs=2, space="PSUM") as c_ps:

                # head-0 weights first so the PE can start a few us in; x^T
                # streams behind them in 4 token-major chunks so head-0's
                # projections unblock chunk by chunk (the warm-up matmuls
                # above cover the first ~9us of DMA).
                wqk0 = wqk_pool.tile([P, DT, 256], BF16, tag="wqk")
                nc.sync.dma_start(out=wqk0[:], in_=wqk_t[0])

                xt = xt_pool.tile([P, T // 512, DT, 512], BF16)
                nc.sync.dma_start(out=xt[:, 0], in_=xt_t[:, 0])

                wv0 = wv_pool.tile([P, DT, 512], BF16, tag="wv")
                nc.sync.dma_start(out=wv0[:], in_=wv_t[0])

                for tch in range(1, T // 512):
                    nc.sync.dma_start(out=xt[:, tch], in_=xt_t[:, tch])

                copy_state = [0]

                def psum_out(dst, ps):
                    if copy_state[0] % 2 == 0:
                        nc.vector.tensor_copy(dst, ps)
                    else:
                        nc.scalar.copy(dst, ps)
                    copy_state[0] += 1

                def emit_qk_chunk(wqk, qk, half, tch, tag):
                    ps = qkv_ps.tile([P, 512], F32, tag="qps",
                                     name=f"qps{tag}{half}{tch}")
                    for dt_ in range(DT):
                        nc.tensor.matmul(
                            ps[:],
                            wqk[:, dt_, half * P:(half + 1) * P],
                            xt[:, tch, dt_, :],
                            start=(dt_ == 0), stop=(dt_ == DT - 1),
                        )
                    psum_out(qk[:, half, tch * 512:(tch + 1) * 512], ps[:])

                def emit_v_chunk(wv, vg, tt, tag):
                    ps = qkv_ps.tile([P, 512], F32, tag="qps",
                                     name=f"vps{tag}{tt}")
                    for dt_ in range(DT):
                        nc.tensor.matmul(
                            ps[:],
                            xt[:, tt // 4, dt_, (tt % 4) * P:(tt % 4 + 1) * P],
                            wv[:, dt_, :],
                            start=(dt_ == 0), stop=(dt_ == DT - 1),
                        )
                    psum_out(vg[:, tt, :], ps[:])

                vg = None
                wqk_next = wqk0
                for h in range(H):
                    hh = h % G
                    # -- weights for this head (and V group) --
                    wqk = wqk_next
                    if hh == 0:
                        vg = v_pool.tile([P, T // P, 512], BF16, tag="vg")
                        if h == 0:
                            wv = wv0
                        else:
                            wv = wv_pool.tile([P, DT, 512], BF16, tag="wv")
                            nc.sync.dma_start(out=wv[:], in_=wv_t[h // G])

                    # -- Q^T / K^T projection: [d', t] feature-major.
                    # dt-inner: each accumulation chain stays in one PSUM
                    # bank (bank-cycling between consecutive MMs measurably
                    # slows the PE).
                    qk = qk_pool.tile([P, 2, T], BF16, tag="qk")
                    if h == 0:
                        # token-chunk-major so each x^T DMA chunk unblocks
                        # the next slice of projection work immediately.
                        for tch in range(T // 512):
                            emit_qk_chunk(wqk, qk, 0, tch, h)
                            emit_qk_chunk(wqk, qk, 1, tch, h)
                            for tt in range(4 * tch, 4 * tch + 4):
                                emit_v_chunk(wv, vg, tt, h // G)
                    elif h < H - 1:
                        for half in range(2):
                            for tch in range(T // 512):
                                emit_qk_chunk(wqk, qk, half, tch, h)
                        if hh == 0:
                            for tt in range(T // P):
                                emit_v_chunk(wv, vg, tt, h // G)
                    # (the last head's Q/K chunks are emitted inside the
                    # attention weave below — they are its only PE filler)

                    # prefetch the next head's Q|K weights before the
                    # attention block so the DMA overlaps it (the first
                    # Q chunk of head h+1 otherwise stalls on this load).
                    if h + 1 < H:
                        wqk_next = wqk_pool.tile([P, DT, 256], BF16,
                                                 tag="wqk")
                        nc.sync.dma_start(out=wqk_next[:], in_=wqk_t[h + 1])

                    # -- causal attention for the 8 local batches.
                    # k-tile 1 only attends to queries q>=128 (causal skip):
                    # a_t columns [0:256] are k-tile 0 x all q, [256:384]
                    # are k-tile 1 x q in [128, 256).
                    def attn_a(b):
                        """scores + exp + mask -> a_t for batch b."""
                        t0 = b * S
                        ps_s = s_ps.tile([P, S + P], F32)
                        nc.tensor.matmul(
                            ps_s[:, 0:S], qk[:, 1, t0:t0 + P],
                            qk[:, 0, t0:t0 + S], start=True, stop=True)
                        nc.tensor.matmul(
                            ps_s[:, S:S + P], qk[:, 1, t0 + P:t0 + S],
                            qk[:, 0, t0 + P:t0 + S], start=True, stop=True)
                        a_t = a_pool.tile([P, S + P], BF16, tag="at")
                        nc.scalar.activation(a_t[:], ps_s[:], EXP, scale=SCALE)
                        nc.vector.tensor_mul(a_t[:], a_t[:], m2[:])
                        return a_t, ps_s

                    def attn_b(b, a_t, ps_s):
                        """unnormalised ctx^T + denominators for batch b."""
                        t0 = b * S
                        # ctx^T over the two k-tiles; evacuate via ScalarE
                        # immediately (no rb dependency) so the PSUM bank
                        # frees fast — normalisation happens in SBUF later.
                        ps_c = c_ps.tile([P, S], F32)
                        nc.tensor.matmul(
                            ps_c[:], vg[:, 2 * b, hh * P:(hh + 1) * P],
                            a_t[:, 0:S], start=True, stop=False)
                        nc.tensor.matmul(
                            ps_c[:, P:S], vg[:, 2 * b + 1, hh * P:(hh + 1) * P],
                            a_t[:, S:S + P], start=False, stop=True,
                            skip_group_check=True)
                        nc.scalar.copy(ctx_b[:, h, t0:t0 + S], ps_c[:])
                        # denominators via TWO accumulating matmuls with
                        # the ALL-ONES 128x128 mask block as stationary:
                        # every output row equals the column sums, i.e. the
                        # matmul does the partition-broadcast for free (no
                        # GpSimd op, no M=1 col_grp penalty), and the
                        # k-tile-1 partials accumulate straight onto the
                        # q>=128 denominators in PSUM — the DVE chain is
                        # just the reciprocal (a congested DVE delays later
                        # pairs' mask-muls and rcp frees this bank late,
                        # stalling the next D).
                        ps_db = db_ps.tile([P, S + P], F32)
                        nc.tensor.matmul(ps_db[:, 0:S], m2[:, P:2 * P],
                                         a_t[:, 0:S],
                                         start=True, stop=False)
                        nc.tensor.matmul(ps_db[:, P:S], m2[:, P:2 * P],
                                         a_t[:, S:S + P],
                                         start=False, stop=True,
                                         skip_group_check=True)
                        rb = r_pool.tile([P, S], F32, tag="rb")
                        nc.vector.reciprocal_approx_fast(
                            rb[:], ps_db[:, 0:S])
                        return rb

                    def attn_norm(b, rb):
                        """ctx_b[h, batch b] *= 1/denom (in SBUF)."""
                        t0 = b * S
                        nc.vector.tensor_mul(ctx_b[:, h, t0:t0 + S],
                                             ctx_b[:, h, t0:t0 + S], rb[:])

                    # Software-pipelined: the scores/exp/mask stage runs 2
                    # batches ahead of the dependent ctx/denominator stage,
                    # so the PE never waits on the ACT+DVE softmax chain;
                    # the normalisation (behind the 0.7us GpSimd broadcast)
                    # trails one more batch, off the critical path.
                    live, norm = [], []

                    def attn_step(b):
                        live.append((b,) + attn_a(b))
                        if len(live) > 2:
                            b0, a0, s0 = live.pop(0)
                            norm.append((b0, attn_b(b0, a0, s0)))
                        while len(norm) > 1:
                            attn_norm(*norm.pop(0))

                    if h < H - 1:
                        for b in range(NB):
                            attn_step(b)
                    else:
                        # last head: no next-head projections exist as PE
                        # filler, so weave the pipeline between this head's
                        # own Q/K chunks (pairs for token block tch only
                        # need the qk chunks of block b//2, so they trail
                        # the chunk loop by one block).
                        for tch in range(T // 512):
                            emit_qk_chunk(wqk, qk, 0, tch, h)
                            emit_qk_chunk(wqk, qk, 1, tch, h)
                            if tch >= 1:
                                attn_step(2 * (tch - 1))
                                attn_step(2 * tch - 1)
                        attn_step(NB - 2)
                        attn_step(NB - 1)
                    for b0, a0, s0 in live:
                        norm.append((b0, attn_b(b0, a0, s0)))
                    for item in norm:
                        attn_norm(*item)

            # ---------------- output projection ----------------
            with tc.tile_pool(name="wo", bufs=2) as wo_pool, \
                 tc.tile_pool(name="p3out", bufs=4) as o3_pool, \
                 tc.tile_pool(name="ps_o", bufs=2, space="PSUM") as o_ps:
                copy_i = 0
                for ec in range(D // 512):
                    wo = wo_pool.tile([P, H, 512], BF16, tag="wo")
                    # split by head group: the first accumulation chain only
                    # needs wo[:, 0:4] — it can start ~1.2us after the DMA
                    # ring picks this up instead of waiting for all 2 MiB.
                    for gq in range(4):
                        nc.sync.dma_start(out=wo[:, 4 * gq:4 * gq + 4, :],
                                          in_=wo_t[ec, :, 4 * gq:4 * gq + 4, :])
                    for tt in range(T // P):
                        ps_o = o_ps.tile([P, 512], F32)
                        for h in range(H):
                            nc.tensor.matmul(
                                ps_o[:],
                                ctx_b[:, h, tt * P:(tt + 1) * P],
                                wo[:, h, :],
                                start=(h == 0), stop=(h == H - 1),
                            )
                        o_t = o3_pool.tile([P, 512], F32, tag="o3")
                        if copy_i % 2 == 0:
                            nc.vector.tensor_copy(o_t[:], ps_o[:])
                        else:
                            nc.scalar.copy(o_t[:], ps_o[:])
                        copy_i += 1
                        nc.sync.dma_start(
                            out=out[tt * P:(tt + 1) * P,
                                    ec * 512:(ec + 1) * 512],
                            in_=o_t[:],
                        )

    nc.compile()
    return nc


def get_nc():
    if "nc" not in _CACHE:
        _CACHE["nc"] = _build()
    return _CACHE["nc"]


def make_in_maps(x, w_qkv, w_o):
    x = np.ascontiguousarray(np.asarray(x, dtype=np.float32))
    w_qkv = np.asarray(w_qkv, dtype=np.float32)
    w_o = np.asarray(w_o, dtype=np.float32)
    bf = ml_dtypes.bfloat16
    # wqk_t [H, P, DT, 256]: [h,p,dt,j<128] = w_qkv[h*128+j, dt*128+p]
    wq = w_qkv[0:D].reshape(H, HD, DT, P).transpose(0, 3, 2, 1)
    wk = w_qkv[D:2 * D].reshape(H, HD, DT, P).transpose(0, 3, 2, 1)
    wqk = np.ascontiguousarray(
        np.concatenate([wq, wk], axis=3)).astype(bf)
    # wv_t [G, P, DT, 512]: [g,p,dt,j] = w_qkv[2D + g*512 + j, dt*128+p]
    wv = np.ascontiguousarray(
        w_qkv[2 * D:].reshape(G, 512, DT, P).transpose(0, 3, 2, 1)).astype(bf)
    # wo_t [EC, P, H, 512]: [ec,p,h,j] = w_o[ec*512+j, h*128+p]
    wo = np.ascontiguousarray(
        w_o.reshape(D // 512, 512, H, HD).transpose(0, 3, 2, 1)).astype(bf)
    # causal mask blocks: [tri(k<=q) | ones | tri]
    tri = np.triu(np.ones((P, P), dtype=np.float32))
    mask2 = np.concatenate(
        [tri, np.ones((P, P), np.float32), tri], axis=1).astype(bf)
    in_maps = []
    for c in range(N_CORES):
        xs = x[c * NB:(c + 1) * NB].reshape(T, D)
        # [P, tch, DT, 512]: [p, tch, dt, j] = xs[tch*512 + j, dt*128 + p]
        xt = np.ascontiguousarray(
            xs.reshape(T // 512, 512, DT, P).transpose(3, 0, 2, 1)).astype(bf)
        in_maps.append({"xt_t": xt, "wqk_t": wqk, "wv_t": wv, "wo_t": wo,
                        "mask2": mask2})
    return in_maps


def run(x, w_qkv, w_o, trace=False):
    nc = get_nc()
    in_maps = make_in_maps(x, w_qkv, w_o)
    res = run_bass_kernel_spmd(nc, in_maps, list(range(N_CORES)), trace=trace)
    outs = [res.results[i]["out"].reshape(NB, S, D) for i in range(N_CORES)]
    return np.concatenate(outs, axis=0), res


def kernel(**inputs):
    out, _ = run(inputs["x"], inputs["w_qkv"], inputs["w_o"])
    return out



# revision 33
# speedup vs baseline: 1.2356x; 1.0021x over previous
"""Trainium2 Bass kernel for nn_Attn_25409026523783.

Dense causal multi-head attention block (B=64, S=256, D=2048, H=16, HD=128):
    qkv = x @ w_qkv.T ; causal softmax attention per head ; out = ctx @ w_o.T

Strategy (fused bf16, zero DRAM spill; PE ~98% busy at the 1 cycle/row
bf16 stream roofline):
  - Batch-shard across the 8 NeuronCores (8 batches / 2048 tokens per core).
    No collectives: host scatters inputs, concatenates per-core outputs.
  - All operands cast to bf16 on the host and pre-tiled so every DMA is a
    contiguous block with the contraction dim on partitions. Matmuls run
    bf16 x bf16 -> fp32 PSUM (1 cycle/row at N>=256, measured same speed as
    fp32r, but half the DMA/SBUF footprint and cheaper LDWEIGHTS).
  - PE warm-up: ~76 junk matmuls on a memset tile (no DMA dependency)
    cover the ~8us DMA-ring spin-up + ~15us first-operand load, so HAM
    un-throttles to 2.4 GHz before the real stream starts.
  - Head-major loop with x^T resident in SBUF (64 KiB/partition bf16,
    loaded in 4 token-major chunks so head 0's projections start early):
    per head, project Q^T/K^T (feature-major) and V (token-major, per
    4-head group), run causal attention for all 8 local batches, and
    write ctx^T into a resident bf16 buffer. Q/K/V never touch DRAM; the
    Tile scheduler interleaves head h's attention with head h+1's
    projections (next head's wqk DMA is prefetched before attention).
  - Attention per (batch, head) in transposed layout S^T[k, q] with causal
    skip (k-tile 1 only computed for queries >= 128), software-pipelined:
    scores/exp(ACT)/mask(DVE) run 2 batches ahead of the dependent work so
    the PE never waits on the softmax chain. Denominators via ONE matmul
    with the all-ones 128x128 mask block as stationary — every output row
    equals the column sums, so the matmul also performs the partition
    broadcast (no GpSimd op, no M=1 col_grp penalty). ctx^T (V x A^T) is
    evacuated unnormalised via ScalarE so its PSUM bank frees immediately;
    a trailing DVE multiply applies 1/den in SBUF off the critical path.
  - The last head has no next-head projections to hide the softmax chain,
    so its attention pipeline is woven between its own Q/K chunk matmuls.
  - Loop-order notes from HW traces: accumulation chains must stay dt-inner
    (one PSUM bank per chain — bank-cycling between consecutive matmuls
    slows the PE ~18%), and gpsimd.partition_all_reduce for the denominators
    is a net loss (~2.8 us/op stalls the pipeline).
  - After the head loop x^T is freed; w_o^T streams in per-512-column
    chunks (split into 4 head-group DMAs so the first output chain starts
    ~1us after the pool opens) and the output projection accumulates ctx^T
    tiles over heads.
"""

import os
import sys

import numpy as np
import ml_dtypes

for _p in ("/opt/trn_rl_repo", "/root/.axon_site/_ro/trn_rl_repo"):
    if os.path.isdir(_p) and _p not in sys.path:
        sys.path.insert(0, _p)

import concourse.bass as bass  # noqa: E402,F401
import concourse.mybir as mybir  # noqa: E402
import concourse.tile as tile  # noqa: E402
from concourse import bacc  # noqa: E402
from concourse.bass_utils import run_bass_kernel_spmd  # noqa: E402


def _ensure_ntff_hook():
    """Some images ship an ``antenv`` without ``axon_hooks``; bass_utils then
    crashes on import when tracing is requested. Provide a no-op-compatible
    module (and register the real ctypes hook when available)."""
    try:
        from antenv import axon_hooks  # noqa: F401
        return
    except ImportError:
        pass
    import types

    mod = types.ModuleType("antenv.axon_hooks")
    mod._hook = None
    mod.set_axon_ntff_profile_hook = lambda h: setattr(mod, "_hook", h)
    mod.get_axon_ntff_profile_hook = lambda: mod._hook
    sys.modules["antenv.axon_hooks"] = mod
    try:
        import antenv

        antenv.axon_hooks = mod
        from trn_agent_boot.trn_boot import _ntff_profile_via_ctypes

        hook = _ntff_profile_via_ctypes("/opt/axon/libaxon_pjrt.so")
        if hook is not None:
            mod._hook = hook
    except Exception:
        pass


_ensure_ntff_hook()

F32 = mybir.dt.float32
F32R = mybir.dt.float32r
BF16 = mybir.dt.bfloat16
EXP = mybir.ActivationFunctionType.Exp

# Problem constants (per spec, hardcoded)
B, S, D, H = 64, 256, 2048, 16
HD = D // H  # 128
N_CORES = 8
NB = B // N_CORES           # 8 batches per core
T = NB * S                  # 2048 tokens per core
P = 128
DT = D // P                 # 16 d-tiles
SCALE = float(HD) ** -0.5
G = 4                       # heads per V-production group

_CACHE = {}


def _build():
    nc = bacc.Bacc("TRN2", target_bir_lowering=False, debug=False,
                   enable_asserts=False)

    # x^T in token-major chunks: [p(d), tch, dt, 512] so each 512-token block
    # is one contiguous DMA and head-0 projections can start after 1/4 of x.
    xt_t = nc.dram_tensor("xt_t", [P, T // 512, DT, 512], BF16,
                          kind="ExternalInput").ap()
    # per-head Q|K weight stripes: [h, p(d), dt, 256] (q cols 0:128, k 128:256)
    wqk_t = nc.dram_tensor("wqk_t", [H, P, DT, 256], BF16,
                           kind="ExternalInput").ap()
    # V weight stripes per 4-head group: [g, p(d), dt, 512]
    wv_t = nc.dram_tensor("wv_t", [G, P, DT, 512], BF16,
                          kind="ExternalInput").ap()
    # w_o^T stripes per 512-col output chunk: [ec, p(d'), h, 512]
    wo_t = nc.dram_tensor("wo_t", [D // 512, P, H, 512], BF16,
                          kind="ExternalInput").ap()
    # mask2 [128, 384]: [tri(k<=q) | ones | tri] (causal-skip layout)
    mask2 = nc.dram_tensor("mask2", [P, 3 * P], BF16,
                           kind="ExternalInput").ap()
    out = nc.dram_tensor("out", [T, D], F32, kind="ExternalOutput").ap()

    with tile.TileContext(nc) as tc:
        with tc.tile_pool(name="const", bufs=1) as c_pool, \
             tc.tile_pool(name="ctx", bufs=1) as ctx_pool:

            m2 = c_pool.tile([P, 3 * P], BF16)
            nc.sync.dma_start(out=m2[:], in_=mask2)
            ones_col = m2[:, 2 * P - 1:2 * P]   # all-ones [128, 1] bf16
            ones_row = m2[0:1, P:2 * P]         # all-ones [1, 128] bf16

            # resident ctx^T accumulator: [128 d', 16 h, 2048 t] bf16
            ctx_b = ctx_pool.tile([P, H, T], BF16, name="ctx_b")

            # PE warm-up: throwaway matmuls on a memset junk tile (no DMA
            # dependency — the DMA rings take ~8us to start moving data and
            # the first real chain needs ~3MB in SBUF, i.e. ~15us). This
            # keeps the PE busy from ~0.5us so HAM un-throttles early and
            # the real stream starts at full 2.4 GHz with zero idle.
            junk = c_pool.tile([P, 384], BF16, name="warm_junk")
            nc.gpsimd.memset(junk[:], 0)
            with tc.tile_pool(name="warm", bufs=1, space="PSUM") as w_ps:
                wps = w_ps.tile([P, 384], F32)
                for _ in range(60):
                    nc.tensor.matmul(wps[:], junk[:, 0:P], junk[:],
                                     start=True, stop=True)

            # ---------------- head loop: QKV + attention ----------------
            with tc.tile_pool(name="xt", bufs=1) as xt_pool, \
                 tc.tile_pool(name="wqk", bufs=2) as wqk_pool, \
                 tc.tile_pool(name="wv", bufs=1) as wv_pool, \
                 tc.tile_pool(name="qk", bufs=2) as qk_pool, \
                 tc.tile_pool(name="vg", bufs=1) as v_pool, \
                 tc.tile_pool(name="at", bufs=4) as a_pool, \
                 tc.tile_pool(name="rcp", bufs=3) as r_pool, \
                 tc.tile_pool(name="ps_qkv", bufs=2, space="PSUM") as qkv_ps, \
                 tc.tile_pool(name="ps_s", bufs=2, space="PSUM") as s_ps, \
                 tc.tile_pool(name="ps_db", bufs=2, space="PSUM") as db_ps, \
                 tc.tile_pool(name="ps_c", bufs=2, space="PSUM") as c_ps:

                # head-0 weights first so the PE can start a few us in; x^T
                # streams behind them in 4 token-major chunks so head-0's
                # projections unblock chunk by chunk (the warm-up matmuls
                # above cover the first ~9us of DMA).
                wqk0 = wqk_pool.tile([P, DT, 256], BF16, tag="wqk")
                nc.sync.dma_start(out=wqk0[:], in_=wqk_t[0])

                xt = xt_pool.tile([P, T // 512, DT, 512], BF16)
                nc.sync.dma_start(out=xt[:, 0], in_=xt_t[:, 0])

                wv0 = wv_pool.tile([P, DT, 512], BF16, tag="wv")
                nc.sync.dma_start(out=wv0[:], in_=wv_t[0])

                for tch in range(1, T // 512):
                    nc.sync.dma_start(out=xt[:, tch], in_=xt_t[:, tch])

                copy_state = [0]

                def psum_out(dst, ps):
                    if copy_state[0] % 2 == 0:
                        nc.vector.tensor_copy(dst, ps)
                    else:
                        nc.scalar.copy(dst, ps)
                    copy_state[0] += 1

                def emit_qk_chunk(wqk, qk, half, tch, tag):
                    ps = qkv_ps.tile([P, 512], F32, tag="qps",
                                     name=f"qps{tag}{half}{tch}")
                    for dt_ in range(DT):
                        nc.tensor.matmul(
                            ps[:],
                            wqk[:, dt_, half * P:(half + 1) * P],
                            xt[:, tch, dt_, :],
                            start=(dt_ == 0), stop=(dt_ == DT - 1),
                        )
                    psum_out(qk[:, half, tch * 512:(tch + 1) * 512], ps[:])

                def emit_v_chunk(wv, vg, tt, tag):
                    ps = qkv_ps.tile([P, 512], F32, tag="qps",
                                     name=f"vps{tag}{tt}")
                    for dt_ in range(DT):
                        nc.tensor.matmul(
                            ps[:],
                            xt[:, tt // 4, dt_, (tt % 4) * P:(tt % 4 + 1) * P],
                            wv[:, dt_, :],
                            start=(dt_ == 0), stop=(dt_ == DT - 1),
                        )
                    psum_out(vg[:, tt, :], ps[:])

                vg = None
                wqk_next = wqk0
                for h in range(H):
                    hh = h % G
                    # -- weights for this head (and V group) --
                    wqk = wqk_next
                    if hh == 0:
                        vg = v_pool.tile([P, T // P, 512], BF16, tag="vg")
                        if h == 0:
                            wv = wv0
                        else:
                            wv = wv_pool.tile([P, DT, 512], BF16, tag="wv")
                            nc.sync.dma_start(out=wv[:], in_=wv_t[h // G])

                    # -- Q^T / K^T projection: [d', t] feature-major.
                    # dt-inner: each accumulation chain stays in one PSUM
                    # bank (bank-cycling between consecutive MMs measurably
                    # slows the PE).
                    qk = qk_pool.tile([P, 2, T], BF16, tag="qk")
                    if h == 0:
                        # token-chunk-major so each x^T DMA chunk unblocks
                        # the next slice of projection work immediately.
                        for tch in range(T // 512):
                            emit_qk_chunk(wqk, qk, 0, tch, h)
                            emit_qk_chunk(wqk, qk, 1, tch, h)
                            for tt in range(4 * tch, 4 * tch + 4):
                                emit_v_chunk(wv, vg, tt, h // G)
                    elif h < H - 1:
                        for half in range(2):
                            for tch in range(T // 512):
                                emit_qk_chunk(wqk, qk, half, tch, h)
                        if hh == 0:
                            for tt in range(T // P):
                                emit_v_chunk(wv, vg, tt, h // G)
                    # (the last head's Q/K chunks are emitted inside the
                    # attention weave below — they are its only PE filler)

                    # prefetch the next head's Q|K weights before the
                    # attention block so the DMA overlaps it (the first
                    # Q chunk of head h+1 otherwise stalls on this load).
                    if h + 1 < H:
                        wqk_next = wqk_pool.tile([P, DT, 256], BF16,
                                                 tag="wqk")
                        nc.sync.dma_start(out=wqk_next[:], in_=wqk_t[h + 1])

                    # -- causal attention for the 8 local batches.
                    # k-tile 1 only attends to queries q>=128 (causal skip):
                    # a_t columns [0:256] are k-tile 0 x all q, [256:384]
                    # are k-tile 1 x q in [128, 256).
                    def attn_a(b):
                        """scores + exp + mask -> a_t for batch b."""
                        t0 = b * S
                        ps_s = s_ps.tile([P, S + P], F32)
                        nc.tensor.matmul(
                            ps_s[:, 0:S], qk[:, 1, t0:t0 + P],
                            qk[:, 0, t0:t0 + S], start=True, stop=True)
                        nc.tensor.matmul(
                            ps_s[:, S:S + P], qk[:, 1, t0 + P:t0 + S],
                            qk[:, 0, t0 + P:t0 + S], start=True, stop=True)
                        a_t = a_pool.tile([P, S + P], BF16, tag="at")
                        nc.scalar.activation(a_t[:], ps_s[:], EXP, scale=SCALE)
                        nc.vector.tensor_mul(a_t[:], a_t[:], m2[:])
                        return a_t, ps_s

                    def attn_b(b, a_t, ps_s):
                        """ctx^T + denominators + normalise for batch b."""
                        t0 = b * S
                        # ctx^T over the two k-tiles
                        ps_c = c_ps.tile([P, S], F32)
                        nc.tensor.matmul(
                            ps_c[:], vg[:, 2 * b, hh * P:(hh + 1) * P],
                            a_t[:, 0:S], start=True, stop=False)
                        nc.tensor.matmul(
                            ps_c[:, P:S], vg[:, 2 * b + 1, hh * P:(hh + 1) * P],
                            a_t[:, S:S + P], start=False, stop=True,
                            skip_group_check=True)
                        # denominators via TWO accumulating matmuls with
                        # the ALL-ONES 128x128 mask block as stationary:
                        # every output row equals the column sums, i.e. the
                        # matmul does the partition-broadcast for free (no
                        # GpSimd op, no M=1 col_grp penalty), and the
                        # k-tile-1 partials accumulate straight onto the
                        # q>=128 denominators in PSUM — no DVE fix-up ops
                        # (a congested DVE delays later pairs' mask-muls
                        # and stalls the PE).
                        ps_db = db_ps.tile([P, S + P], F32)
                        nc.tensor.matmul(ps_db[:, 0:S], m2[:, P:2 * P],
                                         a_t[:, 0:S],
                                         start=True, stop=False)
                        nc.tensor.matmul(ps_db[:, P:S], m2[:, P:2 * P],
                                         a_t[:, S:S + P],
                                         start=False, stop=True,
                                         skip_group_check=True)
                        rb = r_pool.tile([P, S], F32, tag="rb")
                        nc.vector.reciprocal_approx_fast(
                            rb[:], ps_db[:, 0:S])
                        # single fused evacuate+normalise: rb lands right
                        # after the D matmuls now, so coupling the ctx bank
                        # release to it costs nothing and saves the extra
                        # ScalarE copy + second ctx pass.
                        nc.vector.tensor_mul(ctx_b[:, h, t0:t0 + S],
                                             ps_c[:], rb[:])
                        return rb

                    # Software-pipelined: the scores/exp/mask stage runs 2
                    # batches ahead of the dependent ctx/denominator stage,
                    # so the PE never waits on the ACT+DVE softmax chain.
                    live = []

                    def attn_step(b):
                        live.append((b,) + attn_a(b))
                        if len(live) > 2:
                            b0, a0, s0 = live.pop(0)
                            attn_b(b0, a0, s0)

                    if h < H - 1:
                        for b in range(NB):
                            attn_step(b)
                    else:
                        # last head: no next-head projections exist as PE
                        # filler, so weave the pipeline between this head's
                        # own Q/K chunks (pairs for token block tch only
                        # need the qk chunks of block b//2, so they trail
                        # the chunk loop by one block).
                        for tch in range(T // 512):
                            emit_qk_chunk(wqk, qk, 0, tch, h)
                            emit_qk_chunk(wqk, qk, 1, tch, h)
                            if tch >= 1:
                                attn_step(2 * (tch - 1))
                                attn_step(2 * tch - 1)
                        attn_step(NB - 2)
                        attn_step(NB - 1)
                    for b0, a0, s0 in live:
                        attn_b(b0, a0, s0)
                    live = []

            # ---------------- output projection ----------------
            with tc.tile_pool(name="wo", bufs=2) as wo_pool, \
                 tc.tile_pool(name="p3out", bufs=4) as o3_pool, \
                 tc.tile_pool(name="ps_o", bufs=2, space="PSUM") as o_ps:
                copy_i = 0
                for ec in range(D // 512):
                    wo = wo_pool.tile([P, H, 512], BF16, tag="wo")
                    # split by head group: the first accumulation chain only
                    # needs wo[:, 0:4] — it can start ~1.2us after the DMA
                    # ring picks this up instead of waiting for all 2 MiB.
                    for gq in range(4):
                        nc.sync.dma_start(out=wo[:, 4 * gq:4 * gq + 4, :],
                                          in_=wo_t[ec, :, 4 * gq:4 * gq + 4, :])
                    for tt in range(T // P):
                        ps_o = o_ps.tile([P, 512], F32)
                        for h in range(H):
                            nc.tensor.matmul(
                                ps_o[:],
                                ctx_b[:, h, tt * P:(tt + 1) * P],
                                wo[:, h, :],
                                start=(h == 0), stop=(h == H - 1),
                            )
                        o_t = o3_pool.tile([P, 512], F32, tag="o3")
                        if copy_i % 2 == 0:
                            nc.vector.tensor_copy(o_t[:], ps_o[:])
                        else:
                            nc.scalar.copy(o_t[:], ps_o[:])
                        copy_i += 1
                        nc.sync.dma_start(
                            out=out[tt * P:(tt + 1) * P,
                                    ec * 512:(ec + 1) * 512],
                            in_=o_t[:],
                        )

    nc.compile()
    return nc


def get_nc():
    if "nc" not in _CACHE:
        _CACHE["nc"] = _build()
    return _CACHE["nc"]


def make_in_maps(x, w_qkv, w_o):
    x = np.ascontiguousarray(np.asarray(x, dtype=np.float32))
    w_qkv = np.asarray(w_qkv, dtype=np.float32)
    w_o = np.asarray(w_o, dtype=np.float32)
    bf = ml_dtypes.bfloat16
    # wqk_t [H, P, DT, 256]: [h,p,dt,j<128] = w_qkv[h*128+j, dt*128+p]
    wq = w_qkv[0:D].reshape(H, HD, DT, P).transpose(0, 3, 2, 1)
    wk = w_qkv[D:2 * D].reshape(H, HD, DT, P).transpose(0, 3, 2, 1)
    wqk = np.ascontiguousarray(
        np.concatenate([wq, wk], axis=3)).astype(bf)
    # wv_t [G, P, DT, 512]: [g,p,dt,j] = w_qkv[2D + g*512 + j, dt*128+p]
    wv = np.ascontiguousarray(
        w_qkv[2 * D:].reshape(G, 512, DT, P).transpose(0, 3, 2, 1)).astype(bf)
    # wo_t [EC, P, H, 512]: [ec,p,h,j] = w_o[ec*512+j, h*128+p]
    wo = np.ascontiguousarray(
        w_o.reshape(D // 512, 512, H, HD).transpose(0, 3, 2, 1)).astype(bf)
    # causal mask blocks: [tri(k<=q) | ones | tri]
    tri = np.triu(np.ones((P, P), dtype=np.float32))
    mask2 = np.concatenate(
        [tri, np.ones((P, P), np.float32), tri], axis=1).astype(bf)
    in_maps = []
    for c in range(N_CORES):
        xs = x[c * NB:(c + 1) * NB].reshape(T, D)
        # [P, tch, DT, 512]: [p, tch, dt, j] = xs[tch*512 + j, dt*128 + p]
        xt = np.ascontiguousarray(
            xs.reshape(T // 512, 512, DT, P).transpose(3, 0, 2, 1)).astype(bf)
        in_maps.append({"xt_t": xt, "wqk_t": wqk, "wv_t": wv, "wo_t": wo,
                        "mask2": mask2})
    return in_maps


def run(x, w_qkv, w_o, trace=False):
    nc = get_nc()
    in_maps = make_in_maps(x, w_qkv, w_o)
    res = run_bass_kernel_spmd(nc, in_maps, list(range(N_CORES)), trace=trace)
    outs = [res.results[i]["out"].reshape(NB, S, D) for i in range(N_CORES)]
    return np.concatenate(outs, axis=0), res


def kernel(**inputs):
    out, _ = run(inputs["x"], inputs["w_qkv"], inputs["w_o"])
    return out

